# revision 2
# baseline (speedup 1.0000x reference)
"""Multi-head causal attention (B=4, S=2048, D=1024, H=16) on 8 trn2 cores.

Sharding: data-parallel over batch (4) x tensor-parallel over head groups
(2 x 8 heads).  Core c = (b, g).

Per-core pipeline (all matmul costs in "moving rows"):
  Q/K proj: 2-term fp8 DoubleRow  (Wh.Xh + Wh.Xl), psum/32 + 8b -> fp8 qT8/kT8
            layout [128 = 4 heads x 32 hd, 2 hd-slabs, S] per 4-head group
  V proj:   3-term fp8 DoubleRow, psum/256 + bv -> bf16 v4 (+ones col)
  mm1:      fp8 DoubleRow 32-part slabs: S^T[k,q] psum, 2 heads per psum tile
            diagonal causal mask added via ident8 x tri8 matmul (-25 logits)
  exp:      one activation per chunk [128, 2*cnt*128] psum -> bf16 aT2
            PSUM rings A(4 banks, 8 slabs/head) / B(2 banks, 4 slabs)
  mm2:      bf16 aT2 x v4 -> po (both heads one bank), deferred one call
  norm:     reciprocal(DVE) + scale on GPSIMD -> o_nat bf16
  out:      PE transpose, oT fp8 hi/lo (x4), 3-term DoubleRow with Wo x16,
            psum -> bf16 outT; host sums partials / 64 + bo.
"""

import math

import numpy as np

B, S, D, H = 4, 2048, 1024, 16
HD = D // H          # 64
NCORES = 8
HPC = 8              # heads per core
DM = HPC * HD        # 512
NQT = S // 128       # 16
VROW = HD + 1        # 65

SX = 4.0             # X scale for fp8
SW = 64.0            # W scale (proj)
SQK = 8.0            # Q/K scale for mm1
SO = 4.0             # O scale for outproj
SWO = 16.0           # Wo scale
SI, ST = 80.0, -160.0  # ident/tri scales: SI*|ST| = 12800 -> -25 logits

_CACHE = {}


def _build_program():
    import concourse.mybir as mybir
    import concourse.tile as tile
    from concourse import bacc

    f32 = mybir.dt.float32
    bf16 = mybir.dt.bfloat16
    fp8 = mybir.dt.float8e4
    DR = mybir.MatmulPerfMode.DoubleRow
    EXP = mybir.ActivationFunctionType.Exp
    ALU = mybir.AluOpType

    nc = bacc.Bacc("TRN2", target_bir_lowering=False, debug=False,
                   num_devices=NCORES)

    u8 = mybir.dt.uint8
    xd = {t: nc.dram_tensor(f"x{t}", [D, 2 * S], fp8,
                            kind="ExternalInput")
          for t in ("q", "k", "v")}
    wqh_d = nc.dram_tensor("wqh", [D, DM], fp8, kind="ExternalInput")
    wkh_d = nc.dram_tensor("wkh", [D, DM], fp8, kind="ExternalInput")
    wv_d = nc.dram_tensor("wv", [D, 2 * DM], fp8, kind="ExternalInput")
    wo_d = nc.dram_tensor("wo", [DM, 2 * D], fp8, kind="ExternalInput")
    bvb_d = nc.dram_tensor("bvb", [128, DM], f32, kind="ExternalInput")
    # packed consts: identT bf16 | ident8 fp8 | tri8 fp8 | bq8 f32 | bk8 f32
    const_d = nc.dram_tensor("consts", [128, 800], u8,
                             kind="ExternalInput")
    outT_d = nc.dram_tensor("outT", [D, S], bf16, kind="ExternalOutput")

    with tile.TileContext(nc) as tc:
        with (
            tc.tile_pool(name="res", bufs=1) as res,
            tc.tile_pool(name="wrk", bufs=1) as wrk,
            tc.tile_pool(name="ps", bufs=1, space="PSUM") as ps,
        ):
            # ---- resident tiles --------------------------------------
            wqh_sb = res.tile([128, 8 * DM], fp8, tag="wqh")
            wkh_sb = res.tile([128, 8 * DM], fp8, tag="wkh")
            wv_sb = res.tile([128, 2 * 8 * DM], fp8, tag="wv_sb")
            wo_sb = res.tile([128, 2 * 4 * D], fp8, tag="wo_sb")
            bvb_sb = res.tile([128, DM], f32, tag="bvb")
            const_sb = res.tile([128, 800], u8, tag="const_sb")
            identT_sb = const_sb[:, 0:256].bitcast(bf16)
            ident8_sb = const_sb[:, 256:512].bitcast(fp8)
            tri8_sb = const_sb[:, 512:768].bitcast(fp8)
            bq8_sb = const_sb[:, 768:784].bitcast(f32)
            bk8_sb = const_sb[:, 784:800].bitcast(f32)
            # Q/K fp8: per 4-head group [128 = 4h x 32, 2 hd-slabs, S]
            q8_sb = [res.tile([128, 2 * S], fp8, tag=f"q8_{g}", name=f"q8_{g}")
                     for g in range(2)]
            k8_sb = [res.tile([128, 2 * S], fp8, tag=f"k8_{g}", name=f"k8_{g}")
                     for g in range(2)]
            v_sb = res.tile([128, NQT * HPC * VROW], bf16, tag="v_sb")
            oTh_sb = res.tile([128, 4 * S], fp8, tag="oTh")
            oTl_sb = res.tile([128, 4 * S], fp8, tag="oTl")

            q8v = [t.rearrange("p (two s) -> p two s", two=2) for t in q8_sb]
            k8v = [t.rearrange("p (two s) -> p two s", two=2) for t in k8_sb]
            v4 = v_sb.rearrange("p (s h c) -> p s h c", h=HPC, c=VROW)
            bvb3 = bvb_sb.rearrange("p (h c) -> p h c", h=HPC)
            oThv = oTh_sb.rearrange("p (m s) -> p m s", m=4)
            oTlv = oTl_sb.rearrange("p (m s) -> p m s", m=4)
            id8v = ident8_sb.rearrange("p (two f) -> p two f", two=2)
            tri8v = tri8_sb.rearrange("p (two f) -> p two f", two=2)

            wq4 = wqh_sb.rearrange("p (k n) -> p k n", k=8)
            wk4 = wkh_sb.rearrange("p (k n) -> p k n", k=8)
            wvb = wv_sb.rearrange("p (hl k n) -> p hl k n", hl=2, k=8)
            wvh4, wvl4 = wvb[:, 0], wvb[:, 1]
            wob = wo_sb.rearrange("p (hl k n) -> p hl k n", hl=2, k=4)
            woh4, wol4 = wob[:, 0], wob[:, 1]

            def load_w(w_sb, w_d, n_w, k):
                sv = w_sb.rearrange("p (k n) -> p k n", n=n_w)
                dv = w_d.rearrange("(k p) n -> p k n", p=128)
                nc.sync.dma_start(sv, dv)

            # ---- staging loads ---------------------------------------
            def load_xch(tag, n):
                xch = wrk.tile([128, 2 * 8 * 512], fp8, tag=f"x{tag}ch",
                               name=f"x{tag}ch", bufs=3)
                xv_ = xch.rearrange("p (hl k s) -> p hl k s", hl=2, k=8)
                dv = xd[tag].rearrange("(k p) (hl s) -> p hl k s",
                                       p=128, hl=2)[
                    :, :, :, n * 512:(n + 1) * 512]
                nc.sync.dma_start(xv_, dv)
                return xv_  # [p, hi/lo, k, 512]

            # ---- projections -----------------------------------------
            def proj_qk_unit(xv_, w4, b_sb, dstv, n, m):
                # 2-term: Wh.Xh + Wh.Xl ; psum = 256 * Q^T chunk
                pp = ps.tile([128, 512], f32, tag="ps_s", name="pp", bufs=2)
                nmm = 8
                i = 0
                for kt2 in range(4):
                    for xi in range(2):
                        nc.tensor.matmul(
                            pp[:],
                            w4[:, 2 * kt2:2 * kt2 + 2,
                               m * 128:(m + 1) * 128],
                            xv_[:, xi, 2 * kt2:2 * kt2 + 2, :],
                            start=(i == 0), stop=(i == nmm - 1),
                            perf_mode=DR, skip_group_check=True)
                        i += 1
                # qT8[g][:, slab, ncols] = pp/32 + 8*bias
                nc.vector.tensor_scalar(
                    dstv[m // 2][:, m % 2, n * 512:(n + 1) * 512],
                    pp[:], 1.0 / 32, b_sb[:, m:m + 1],
                    op0=ALU.mult, op1=ALU.add)

            def proj_v_unit(xv_, n, mi):
                # 3-term: Xh.Wh + Xl.Wh + Xh.Wl ; psum = 256 * V chunk
                st = n * 4 + mi
                pp = ps.tile([128, 512], f32, tag="ps_s", name="pp", bufs=2)
                terms = ((0, wvh4), (1, wvh4), (0, wvl4))
                i = 0
                for kt2 in range(4):
                    for (xi, wv4_) in terms:
                        nc.tensor.matmul(
                            pp[:],
                            xv_[:, xi, 2 * kt2:2 * kt2 + 2,
                                mi * 128:(mi + 1) * 128],
                            wv4_[:, 2 * kt2:2 * kt2 + 2, :],
                            start=(i == 0), stop=(i == 11),
                            perf_mode=DR, skip_group_check=True)
                        i += 1
                nc.vector.scalar_tensor_tensor(
                    v4[:, st, :, 0:HD],
                    pp.rearrange("p (h c) -> p h c", h=HPC),
                    1.0 / 256, bvb3[:],
                    op0=ALU.mult, op1=ALU.add)

            xcache = {}

            def load_chunk(n):
                xcache[n] = (load_xch("q", n), load_xch("k", n),
                             load_xch("v", n))

            def proj_units(n):
                # units for chunk n from xcache[n]; as the last unit pops,
                # kick off chunk n+1's X DMAs (one phase ahead of its use)
                for m in range(4):
                    yield lambda m=m: proj_qk_unit(xcache[n][0], wq4,
                                                   bq8_sb, q8v, n, m)
                for m in range(4):
                    yield lambda m=m: proj_qk_unit(xcache[n][1], wk4,
                                                   bk8_sb, k8v, n, m)
                for mi in range(4):
                    yield lambda mi=mi: proj_v_unit(xcache[n][2], n, mi)
                if n < 3:
                    yield lambda: load_chunk(n + 1)

            # ---- output projection -----------------------------------
            outTm = outT_d.rearrange("(m p) s -> p m s", p=128)

            def outproj_unit(n, m8):
                # 3-term: Woh.Oh + Woh.Ol + Wol.Oh over DM = 2 pairs
                c0 = n * 512
                pp = ps.tile([128, 512], f32, tag="ps_s", name="pp", bufs=2)
                terms = ((woh4, oThv), (woh4, oTlv), (wol4, oThv))
                i = 0
                for kt2 in range(2):
                    for (wv_, ov_) in terms:
                        nc.tensor.matmul(
                            pp[:],
                            wv_[:, 2 * kt2:2 * kt2 + 2,
                                m8 * 128:(m8 + 1) * 128],
                            ov_[:, 2 * kt2:2 * kt2 + 2, c0:c0 + 512],
                            start=(i == 0), stop=(i == 5),
                            perf_mode=DR, skip_group_check=True)
                        i += 1
                ost = wrk.tile([128, 512], bf16, tag="ost", name="ost",
                               bufs=2)
                nc.vector.tensor_copy(ost[:], pp[:])
                nc.sync.dma_start(
                    outT_d[m8 * 128:(m8 + 1) * 128, c0:c0 + 512], ost[:])

            def outproj_unit_q(qt, half):
                # q-granular super-unit: 4 m8 blocks in one psum bank;
                # start=True only on the first matmul (whole-bank zero),
                # each later block's first write lands on pending-zero.
                c0 = qt * 128
                pp = ps.tile([128, 512], f32, tag="ps_s", name="pp", bufs=2)
                terms = ((woh4, oThv), (woh4, oTlv), (wol4, oThv))
                first = True
                for mi in range(4):
                    m8 = half * 4 + mi
                    i = 0
                    for kt2 in range(2):
                        for (wv_, ov_) in terms:
                            nc.tensor.matmul(
                                pp[:, mi * 128:(mi + 1) * 128],
                                wv_[:, 2 * kt2:2 * kt2 + 2,
                                    m8 * 128:(m8 + 1) * 128],
                                ov_[:, 2 * kt2:2 * kt2 + 2, c0:c0 + 128],
                                start=first, stop=(i == 5),
                                perf_mode=DR, skip_group_check=True)
                            first = False
                            i += 1
                ost = wrk.tile([128, 512], bf16, tag="ost", name="ost",
                               bufs=2)
                nc.vector.tensor_copy(ost[:], pp[:])
                nc.sync.dma_start(
                    outTm[:, half * 4:(half + 1) * 4, c0:c0 + 128],
                    ost.rearrange("p (m c) -> p m c", m=4))

            # ---- attention -------------------------------------------
            parity = [0]

            def emit_mm1(qt, hp, ring, c0, cnt):
                heads = (2 * hp, 2 * hp + 1)
                cap = 6
                half = cap * 128
                psx = ps.tile([128, 2 * half], f32,
                              tag=f"ps_x{ring}", name=f"psx{ring}",
                              bufs=1)
                for j in range(cnt):
                    kt = c0 + j
                    diag = (kt == qt)
                    for hh in range(2):
                        h = heads[hh]
                        g4, b32 = h // 4, 32 * (h % 4)
                        slab = psx[:, hh * half + j * 128:
                                   hh * half + (j + 1) * 128]
                        nc.tensor.matmul(
                            slab,
                            k8v[g4][b32:b32 + 32, :,
                                    kt * 128:(kt + 1) * 128],
                            q8v[g4][b32:b32 + 32, :,
                                    qt * 128:(qt + 1) * 128],
                            start=True, stop=not diag,
                            perf_mode=DR, skip_group_check=True,
                            tile_position=(b32, 0))
                        if diag:
                            nc.tensor.matmul(
                                slab, id8v[0:64], tri8v[0:64],
                                start=False, stop=True,
                                perf_mode=DR, skip_group_check=True)
                return psx

            def emit_exp(psx, aT2v, ring, c0, cnt):
                cap = 6
                nc.scalar.activation(
                    aT2v[:, :, c0 * 128:(c0 + cnt) * 128],
                    psx.rearrange("p (hh s) -> p hh s", hh=2)[
                        :, :, 0:cnt * 128],
                    EXP, scale=1.0 / 512)

            def chunk_plan(qt):
                nblk = qt + 1
                c0 = 0
                out = []
                while c0 < nblk:
                    ring = parity[0]
                    parity[0] ^= 1
                    cnt = min(6, nblk - c0)
                    out.append((ring, c0, cnt))
                    c0 += cnt
                return out

            def mm2_tail(qt, hp, aT2v, o_nat):
                nblk = qt + 1
                heads = (2 * hp, 2 * hp + 1)
                po = ps.tile([128, 512], f32, tag="ps_s", name="po",
                             bufs=2)
                for kt in range(nblk):
                    for hh in range(2):
                        nc.tensor.matmul(
                            po[:, hh * VROW:hh * VROW + VROW],
                            aT2v[:, hh, kt * 128:(kt + 1) * 128],
                            v4[:, kt, heads[hh], :],
                            start=(kt == 0 and hh == 0),
                            stop=(kt == nblk - 1),
                            skip_group_check=True)
                po3 = po[:, 0:2 * VROW].rearrange("p (b c) -> p b c", c=VROW)
                rc = wrk.tile([128, 2], f32, tag="rc", name="rc", bufs=3)
                nc.vector.reciprocal(rc[:], po3[:, :, HD:HD + 1])
                rcb = rc.rearrange("p (b o) -> p b o", o=1).to_broadcast(
                    (128, 2, HD))
                nc.vector.tensor_tensor(
                    o_nat[:, hp * 128:(hp + 1) * 128].rearrange(
                        "p (b c) -> p b c", b=2),
                    po3[:, :, 0:HD], rcb, op=ALU.mult)

            def transpose_o(qt, o_nat):
                pt = ps.tile([128, 512], bf16, tag="ps_s", name="pt",
                             bufs=2)
                for m in range(4):
                    nc.tensor.transpose(
                        pt[:, m * 128:(m + 1) * 128],
                        o_nat[:, m * 128:(m + 1) * 128],
                        identT_sb[:])
                ptv = pt.rearrange("p (m s) -> p m s", m=4)
                nc.vector.tensor_scalar(
                    oThv[:, :, qt * 128:(qt + 1) * 128], ptv[:],
                    SO, None, op0=ALU.mult)
                nc.vector.scalar_tensor_tensor(
                    oTlv[:, :, qt * 128:(qt + 1) * 128], ptv[:],
                    SO, oThv[:, :, qt * 128:(qt + 1) * 128],
                    op0=ALU.mult, op1=ALU.subtract)

            # ---- phase 0: DMAs ordered by first need; PE warmed up ---
            nc.sync.dma_start(const_sb[:], const_d[:])
            load_w(wqh_sb, wqh_d, DM, 8)
            xq0 = load_xch("q", 0)
            load_w(wkh_sb, wkh_d, DM, 8)
            xk0 = load_xch("k", 0)
            # p-state warmup: ~3us of junk transposes so real matmuls
            # start at full clock (runs while the X/W DMAs transfer)
            for _ in range(30):
                wu = ps.tile([128, 512], bf16, tag="ps_s", name="wu",
                             bufs=2)
                nc.tensor.transpose(wu[:, 0:128], identT_sb[:],
                                    identT_sb[:])
            for m in range(2):
                proj_qk_unit(xq0, wq4, bq8_sb, q8v, 0, m)
            for m in range(2):
                proj_qk_unit(xk0, wk4, bk8_sb, k8v, 0, m)
            nc.sync.dma_start(bvb_sb[:], bvb_d[:])
            xv0 = load_xch("v", 0)
            xcache[0] = (xq0, xk0, xv0)
            nc.sync.dma_start(
                wvb, wv_d.rearrange("(k p) (hl n) -> p hl k n",
                                    p=128, hl=2))
            nc.gpsimd.memset(v4[:, :, :, HD:HD + 1], 1.0)
            load_chunk(1)
            nc.sync.dma_start(
                wob, wo_d.rearrange("(k p) (hl n) -> p hl k n",
                                    p=128, hl=2))
            # group-1 Q/K units run while the V/wo DMAs are in flight;
            # they must be emitted before qt0's hp=2 mm1 (lookahead)
            for m in range(2, 4):
                proj_qk_unit(xq0, wq4, bq8_sb, q8v, 0, m)
            for m in range(2, 4):
                proj_qk_unit(xk0, wk4, bk8_sb, k8v, 0, m)

            # ---- main loop -------------------------------------------
            # tails: deferred ~2 chunk-periods (ready <- next <- new)
            tails_ready = []
            tails_next = []
            fillers = [lambda mi=mi: proj_v_unit(xv0, 0, mi)
                       for mi in range(4)]

            def pump(n_pop):
                while tails_ready:
                    tails_ready.pop(0)()
                tails_ready.extend(tails_next)
                del tails_next[:]
                for _ in range(n_pop):
                    if fillers:
                        fillers.pop(0)()

            def chunk_stream():
                """Yields (mm1_emit, exp_emit, post_emit) per chunk.
                post_emit runs tail bookkeeping at the chunk boundary."""
                for n in range(4):
                    if n < 3:
                        fillers.extend(proj_units(n + 1))
                    for qt in range(4 * n, 4 * n + 4):
                        o_nat = wrk.tile([128, DM], bf16, tag="o_nat",
                                         name="o_nat", bufs=2)
                        for hp in range(4):
                            aT2 = wrk.tile([128, 2 * S], bf16, tag="aT2",
                                           name="aT2", bufs=3)
                            aT2v = aT2.rearrange("p (hh s) -> p hh s",
                                                 hh=2)
                            plan = chunk_plan(qt)
                            for ci, (ring, c0, cnt) in enumerate(plan):
                                post = []
                                if ci == len(plan) - 1:
                                    post.append(
                                        lambda qt=qt, hp=hp, a=aT2v,
                                        o=o_nat:
                                        tails_next.append(
                                            lambda: mm2_tail(qt, hp,
                                                             a, o)))
                                    if hp == 3:
                                        post.append(
                                            lambda qt=qt, o=o_nat:
                                            tails_next.append(
                                                lambda:
                                                transpose_o(qt, o)))
                                        if qt >= 12:
                                            post.append(
                                                lambda qt=qt:
                                                tails_next.append(
                                                    lambda qt=qt:
                                                    fillers.extend([
                                                        lambda:
                                                        outproj_unit_q(
                                                            qt, 0),
                                                        lambda:
                                                        outproj_unit_q(
                                                            qt, 1)])))
                                        elif qt % 4 == 3:
                                            post.append(
                                                lambda n=n:
                                                tails_next.append(
                                                    lambda n=n:
                                                    fillers.extend(
                                                        lambda n=n,
                                                        m8=m8:
                                                        outproj_unit(
                                                            n, m8)
                                                        for m8 in
                                                        range(8))))
                                yield (
                                    lambda r=ring, c=c0, ct=cnt, q=qt,
                                    h=hp: emit_mm1(q, h, r, c, ct),
                                    lambda psx, r=ring, c=c0, ct=cnt,
                                    a=aT2v: emit_exp(psx, a, r, c, ct),
                                    post,
                                )

            # drive with one-chunk mm1 lookahead: PE sees mm1(j), mm1(j+1),
            # pump(j), mm1(j+2)...  exp(j) deps resolve during exp(j-1).
            it = chunk_stream()
            cur = next(it)
            psx_cur = cur[0]()
            while cur is not None:
                nxt = next(it, None)
                psx_nxt = nxt[0]() if nxt is not None else None
                cur[1](psx_cur)
                for p in cur[2]:
                    p()
                backlog = len(fillers)
                pump(2 if backlog > 10 else 1)
                cur, psx_cur = nxt, psx_nxt
            while tails_ready or tails_next or fillers:
                pump(4)

    nc.compile()
    return nc


def _get_program():
    if "nc" not in _CACHE:
        _CACHE["nc"] = _build_program()
    return _CACHE["nc"]


def _hilo(x, s):
    import ml_dtypes

    FP8 = ml_dtypes.float8_e4m3
    xs = np.asarray(x, np.float32) * s
    hi = xs.astype(FP8)
    lo = (xs - hi.astype(np.float32)).astype(FP8)
    return hi, lo


def _qk_perm():
    """psum partition (m*128+p) -> original W column (per core slice)."""
    perm = np.empty(DM, np.int64)
    for m in range(4):
        for p in range(128):
            perm[m * 128 + p] = ((m // 2) * 256 + (p // 32) * 64
                                 + (m % 2) * 32 + (p % 32))
    return perm


def _make_in_maps(query, key, value, Wq, bq, Wk, bk, Wv, bv, Wo):
    import ml_dtypes

    FP8 = ml_dtypes.float8_e4m3
    BF16 = ml_dtypes.bfloat16
    perm = _qk_perm()

    identT = np.eye(128, dtype=np.float32).astype(BF16)
    ident8 = np.zeros((128, 256), np.float32)
    tri8 = np.zeros((128, 256), np.float32)
    for i in range(2):
        for p in range(64):
            c = 64 * i + p
            ident8[p, i * 128 + c] = SI
            tri8[p, i * 128:i * 128 + c] = ST
    ident8 = ident8.astype(FP8)
    tri8 = tri8.astype(FP8)

    in_maps = []
    for c in range(NCORES):
        b, g = c // 2, c % 2
        sl = slice(g * DM, (g + 1) * DM)
        m = {}
        for t, x in (("q", query), ("k", key), ("v", value)):
            hi, lo = _hilo(np.ascontiguousarray(x[b].T), SX)
            m[f"x{t}"] = np.ascontiguousarray(
                np.concatenate([hi, lo], axis=1))
        wq_s = np.ascontiguousarray(Wq[:, sl][:, perm])
        wk_s = np.ascontiguousarray(Wk[:, sl][:, perm])
        m["wqh"], _ = _hilo(wq_s, SW)
        m["wkh"], _ = _hilo(wk_s, SW)
        wvh, wvl = _hilo(Wv[:, sl], SW)
        m["wv"] = np.ascontiguousarray(np.concatenate([wvh, wvl], axis=1))
        woh, wol = _hilo(Wo[sl, :], SWO)
        m["wo"] = np.ascontiguousarray(np.concatenate([woh, wol], axis=1))
        bq8 = np.ascontiguousarray(
            (SQK * bq[sl][perm]).reshape(4, 128).T.astype(np.float32))
        bk8 = np.ascontiguousarray(
            (SQK * bk[sl][perm]).reshape(4, 128).T.astype(np.float32))
        m["bvb"] = np.ascontiguousarray(
            np.broadcast_to(bv[sl], (128, DM)).astype(np.float32))
        m["consts"] = np.ascontiguousarray(np.concatenate([
            identT.view(np.uint8), ident8.view(np.uint8),
            tri8.view(np.uint8), bq8.view(np.uint8), bk8.view(np.uint8),
        ], axis=1))
        in_maps.append(m)
    return in_maps


def _run_spmd(in_maps, trace=False):
    from concourse import bass_utils

    nc = _get_program()
    return bass_utils.run_bass_kernel_spmd(
        nc, in_maps, core_ids=list(range(NCORES)), trace=trace)


def _assemble(res, bo):
    out = np.empty((B, S, D), dtype=np.float32)
    bo32 = np.asarray(bo, dtype=np.float32)
    for b in range(B):
        out[b] = (res.results[2 * b]["outT"].astype(np.float32).T
                  + res.results[2 * b + 1]["outT"].astype(np.float32).T
                  ) / (SO * SWO) + bo32
    return out


def _numpy_fallback(query, key, value, mask, Wq, bq, Wk, bk, Wv, bv, Wo, bo):
    """Correct host path for non-causal masks."""
    def split_heads(x):
        b, s, _ = x.shape
        return x.reshape(b, s, H, HD).transpose(0, 2, 1, 3)

    q = split_heads(query @ Wq + bq)
    k = split_heads(key @ Wk + bk)
    v = split_heads(value @ Wv + bv)
    nb = query.shape[0]
    out = np.empty((nb, H, S, HD), dtype=np.float32)
    for b in range(nb):
        mb = np.asarray(mask[b, 0]) != 0
        for h in range(H):
            s = (q[b, h] @ k[b, h].T) / math.sqrt(HD)
            s = np.where(mb, s, -np.inf)
            s -= s.max(axis=-1, keepdims=True)
            e = np.exp(s)
            a = e / e.sum(axis=-1, keepdims=True)
            a *= mb
            out[b, h] = a @ v[b, h]
    out = out.transpose(0, 2, 1, 3).reshape(nb, -1, D)
    return (out @ Wo + bo).astype(np.float32)


def kernel(query, key, value, mask, Wq, bq, Wk, bk, Wv, bv, Wo, bo):
    query = np.asarray(query, dtype=np.float32)
    key = np.asarray(key, dtype=np.float32)
    value = np.asarray(value, dtype=np.float32)
    mask = np.asarray(mask)
    Wq = np.asarray(Wq, dtype=np.float32)
    bq = np.asarray(bq, dtype=np.float32)
    Wk = np.asarray(Wk, dtype=np.float32)
    bk = np.asarray(bk, dtype=np.float32)
    Wv = np.asarray(Wv, dtype=np.float32)
    bv = np.asarray(bv, dtype=np.float32)
    Wo = np.asarray(Wo, dtype=np.float32)
    bo = np.asarray(bo, dtype=np.float32)

    causal = np.array_equal(
        np.asarray(mask[0, 0], dtype=np.int32),
        np.tril(np.ones((S, S), dtype=np.int32)),
    ) and all(np.array_equal(mask[b], mask[0])
              for b in range(1, mask.shape[0]))
    if not causal:
        return _numpy_fallback(
            query, key, value, mask, Wq, bq, Wk, bk, Wv, bv, Wo, bo)

    in_maps = _make_in_maps(query, key, value, Wq, bq, Wk, bk, Wv, bv, Wo)
    res = _run_spmd(in_maps)
    return _assemble(res, bo)
